# revision 41
# baseline (speedup 1.0000x reference)
"""Causal self-attention (B=4, T=2048, C=1024, H=16) on 8 trn2 NeuronCores.

Sharding: 8 shards = (batch b in 0..3) x (half-of-heads hh in 0..1).
Each core computes, for its batch b and its 8 heads:
  - Q/K/V projections (bf16 matmuls, fp32 accum), with Q^T/K^T produced in
    transposed [c_out, t] layout so attention needs no on-device transposes
  - scores^T[k, q] = K_h^T.T @ Q_h^T (two heads packed in the 128-partition
    dim via 64-row PE array tiling), exp on ACT (unnormalized softmax),
    causal mask via precomputed multiplicative mask tiles on DVE
  - O^T = V'_h.T @ expS with a ones-column appended to V so the softmax
    denominator Z appears as row 64 of the PSUM accumulator
  - normalize with 1/Z (DVE reciprocal + gpsimd partition_broadcast + DVE mul)
  - partial out-projection y_part = O_norm @ Wo[hh-slice, :]
Projection and out-projection matmuls are interleaved between the QK/AV
matmuls of the previous/next chunk so the PE never idles while ACT computes
exp. Host sums the two partials per batch and adds bo.
"""

import os
import sys

for _p in ("/opt/trn_rl_repo",):
    if _p not in sys.path and os.path.isdir(_p):
        sys.path.insert(0, _p)

from contextlib import ExitStack

import ml_dtypes
import numpy as np

import concourse.bacc as bacc
import concourse.mybir as mybir
import concourse.tile as tile
from concourse import bass_utils

B, T, C, H = 4, 2048, 1024, 16
D = 64                 # head dim
HLOC = 8               # heads per core
CS = HLOC * D          # 512: per-core slice of C on the head axis
SCALE = 1.0 / 8.0      # 1/sqrt(D)
NP = 128               # partitions
QC = 512               # q chunk (PSUM bank width in fp32)
NQC = T // QC          # 4
NKT = T // NP          # 16 k tiles
NCT = C // NP          # 8 contraction tiles for projections
NPAIR = HLOC // 2      # 4 head pairs

F32 = mybir.dt.float32
BF16 = mybir.dt.bfloat16
BF = ml_dtypes.bfloat16

_CACHE = {}


def _build():
    nc = bacc.Bacc(
        "TRN2",
        target_bir_lowering=False,
        debug=False,
        enable_asserts=False,
        num_devices=8,
    )

    qT_d = nc.dram_tensor("qT", [C, T], BF16, kind="ExternalInput").ap()
    kT_d = nc.dram_tensor("kT", [C, T], BF16, kind="ExternalInput").ap()
    vT_d = nc.dram_tensor("vT", [C, T], BF16, kind="ExternalInput").ap()
    Wq_d = nc.dram_tensor("Wq", [C, CS], BF16, kind="ExternalInput").ap()
    Wk_d = nc.dram_tensor("Wk", [C, CS], BF16, kind="ExternalInput").ap()
    Wv_d = nc.dram_tensor("Wv", [C, CS], BF16, kind="ExternalInput").ap()
    Wo_d = nc.dram_tensor("Wo", [CS, C], BF16, kind="ExternalInput").ap()
    bq_d = nc.dram_tensor("bq", [CS], F32, kind="ExternalInput").ap()
    bk_d = nc.dram_tensor("bk", [CS], F32, kind="ExternalInput").ap()
    bv_d = nc.dram_tensor("bv", [CS], BF16, kind="ExternalInput").ap()
    y_d = nc.dram_tensor("y", [T, C], F32, kind="ExternalOutput").ap()

    with tile.TileContext(nc) as tc, ExitStack() as ctx:
        wpool = ctx.enter_context(tc.tile_pool(name="wpool", bufs=1))
        cpool = ctx.enter_context(tc.tile_pool(name="cpool", bufs=1))
        xpool = ctx.enter_context(tc.tile_pool(name="xpool", bufs=2))
        epool = ctx.enter_context(tc.tile_pool(name="epool", bufs=1))
        spool = ctx.enter_context(tc.tile_pool(name="spool", bufs=2))
        ypool = ctx.enter_context(tc.tile_pool(name="ypool", bufs=3))
        psP = ctx.enter_context(tc.tile_pool(name="psP", bufs=2, space="PSUM"))
        psS = ctx.enter_context(tc.tile_pool(name="psS", bufs=2, space="PSUM"))
        psO = ctx.enter_context(tc.tile_pool(name="psO", bufs=2, space="PSUM"))

        # ---- persistent weights / consts ----
        # Weights live in single wide tiles; each load is ONE batched DMA
        # ([1024, n] DRAM -> [128, 8*n] SBUF via a 3D access pattern) to
        # amortize the per-DMA HWDGE overhead. DMA priority order: Wq + qT
        # chunk 0 first so the Q projection can start as early as possible.
        def w_tile_and_dma(name, dram, n_ct, width, dt, halves=1):
            t = wpool.tile([NP, n_ct * width], dt, name=name, tag=name)
            h = n_ct // halves
            for i in range(halves):
                nc.sync.dma_start(
                    t[:, i * h * width : (i + 1) * h * width].rearrange(
                        "p (j n) -> p j n", n=width
                    ),
                    dram[i * h * NP : (i + 1) * h * NP, :].rearrange(
                        "(j p) n -> p j n", p=NP
                    ),
                )
            return t

        # x-stream tiles: one [128, 4096] tile per (input, chunk), loaded in
        # one DMA; bufs=2 per tag keeps two chunks in flight
        xcur = {}

        def emit_x_dma(c, inputs=(0, 1, 2), halves=1):
            for ii in inputs:
                x_d = (qT_d, kT_d, vT_d)[ii]
                xt = xpool.tile([NP, NCT * QC], BF16, name=f"x{ii}", tag=f"x{ii}")
                h = NCT // halves
                for i in range(halves):
                    nc.sync.dma_start(
                        xt[:, i * h * QC : (i + 1) * h * QC].rearrange(
                            "p (j n) -> p j n", n=QC
                        ),
                        x_d[i * h * NP : (i + 1) * h * NP, c * QC : (c + 1) * QC]
                        .rearrange("(j p) n -> p j n", p=NP),
                    )
                xcur[(c, ii)] = xt

        # interleave W/x half-DMAs so the first projection matmuls can start
        # after ~1MB instead of ~2MB has landed (subtile deps gate per-half)
        def w_half_dma(t, dram, width, i, parts=2):
            h = NCT // parts
            nc.sync.dma_start(
                t[:, i * h * width : (i + 1) * h * width].rearrange(
                    "p (j n) -> p j n", n=width
                ),
                dram[i * h * NP : (i + 1) * h * NP, :].rearrange(
                    "(j p) n -> p j n", p=NP
                ),
            )

        def x_half_dma(c, ii, i, parts=2):
            x_d = (qT_d, kT_d, vT_d)[ii]
            h = NCT // parts
            nc.sync.dma_start(
                xcur[(c, ii)][:, i * h * QC : (i + 1) * h * QC].rearrange(
                    "p (j n) -> p j n", n=QC
                ),
                x_d[i * h * NP : (i + 1) * h * NP, c * QC : (c + 1) * QC]
                .rearrange("(j p) n -> p j n", p=NP),
            )

        Wq_sb = wpool.tile([NP, NCT * CS], BF16, name="Wq", tag="Wq")
        Wk_sb = wpool.tile([NP, NCT * CS], BF16, name="Wk", tag="Wk")
        xcur[(0, 0)] = xpool.tile([NP, NCT * QC], BF16, name="x0", tag="x0")
        xcur[(0, 1)] = xpool.tile([NP, NCT * QC], BF16, name="x1", tag="x1")
        bq_sb = cpool.tile([NP, 4], F32, name="bq_sb", tag="bq_sb")
        bk_sb = cpool.tile([NP, 4], F32, name="bk_sb", tag="bk_sb")
        w_half_dma(Wq_sb, Wq_d, CS, 0)
        x_half_dma(0, 0, 0)
        nc.sync.dma_start(bq_sb[:], bq_d.rearrange("(t p) -> p t", p=NP))
        w_half_dma(Wq_sb, Wq_d, CS, 1)
        x_half_dma(0, 0, 1)
        w_half_dma(Wk_sb, Wk_d, CS, 0)
        x_half_dma(0, 1, 0)
        nc.sync.dma_start(bk_sb[:], bk_d.rearrange("(t p) -> p t", p=NP))
        w_half_dma(Wk_sb, Wk_d, CS, 1)
        x_half_dma(0, 1, 1)
        Wv_sb = w_tile_and_dma("Wv", Wv_d, NCT, CS, BF16)
        bv_sb = cpool.tile([NP, CS], BF16, name="bv_sb", tag="bv_sb")
        nc.sync.dma_start(bv_sb[0:1, :], bv_d.rearrange("(o f) -> o f", o=1))
        emit_x_dma(0, inputs=(2,))
        Wo_sb = w_tile_and_dma("Wo", Wo_d, NPAIR, C, BF16)
        ones_sb = cpool.tile([NP, NP], BF16, name="ones_sb", tag="ones_sb")
        nc.any.memset(ones_sb[0:1, :], 1.0)

        # Causal mask for the single partially-masked [128,128] block of each
        # diagonal tile (the staircase: columns below the block are skipped
        # entirely, columns above are fully valid). Duplicated in two halves
        # so both heads mask with one op: mask[p, f%128] = 1.0 if f%128 >= p.
        mask_sb = cpool.tile([NP, 2 * NP], BF16, name="mask_sb", tag="mask_sb")
        nc.gpsimd.memset(mask_sb[:], 1.0)
        nc.gpsimd.affine_select(
            out=mask_sb.rearrange("p (h f) -> p h f", h=2),
            in_=mask_sb.rearrange("p (h f) -> p h f", h=2),
            pattern=[[0, 2], [1, NP]],
            compare_op=mybir.AluOpType.is_ge,
            fill=0.0,
            base=0,
            channel_multiplier=-1,
        )

        # persistent activations
        QT_sb = [
            cpool.tile([NP, T], BF16, name=f"QT{p}", tag=f"QT{p}") for p in range(NPAIR)
        ]
        KT_sb = [
            cpool.tile([NP, T], BF16, name=f"KT{p}", tag=f"KT{p}") for p in range(NPAIR)
        ]
        ON_sb = [
            cpool.tile([NP, T], BF16, name=f"ON{p}", tag=f"ON{p}") for p in range(NPAIR)
        ]
        # V' padded: per k-tile [128, 8 heads * 65], col 64 of each 65-block = 1.0
        V_sb = [
            cpool.tile([NP, HLOC * 65], BF16, name=f"V{t}", tag=f"V{t}")
            for t in range(NKT)
        ]
        for t in range(NKT):
            v3 = V_sb[t].rearrange("p (h e) -> p h e", e=65)
            nc.any.memset(v3[:, :, 64:65], 1.0)

        # ---------------- emission helpers ----------------
        def gen_qk_proj_part(qc, ot):
            """Q^T and K^T projection matmuls for chunk qc, c_out tile ot.
            Yields one closure per PE matmul; epilogue rides on the last."""
            for ii, (W_sb, b_sb, OUT) in enumerate(
                ((Wq_sb, bq_sb, QT_sb), (Wk_sb, bk_sb, KT_sb))
            ):
                ps = psP.tile([NP, QC], F32, name="ps_proj", tag="ps_proj")
                for j in range(NCT):
                    last = j == NCT - 1

                    def mm(ps=ps, ii=ii, W_sb=W_sb, b_sb=b_sb, OUT=OUT, j=j, last=last):
                        nc.tensor.matmul(
                            ps[:],
                            W_sb[:, j * CS + ot * NP : j * CS + (ot + 1) * NP],
                            xcur[(qc, ii)][:, j * QC : (j + 1) * QC],
                            start=(j == 0),
                            stop=last,
                        )
                        if last:
                            nc.vector.tensor_scalar_add(
                                OUT[ot][:, qc * QC : (qc + 1) * QC],
                                ps[:],
                                b_sb[:, ot : ot + 1],
                            )

                    yield mm

        def gen_v_proj_part(qc, ts):
            """V projection matmuls for chunk qc, t-subtile ts."""
            t_tile = qc * 4 + ts
            ps = psP.tile([NP, QC], F32, name="ps_proj", tag="ps_proj")
            for j in range(NCT):

                def mm(ps=ps, j=j):
                    nc.tensor.matmul(
                        ps[:],
                        xcur[(qc, 2)][:, j * QC + ts * NP : j * QC + (ts + 1) * NP],
                        Wv_sb[:, j * CS : (j + 1) * CS],
                        start=(j == 0),
                        stop=False,
                    )

                yield mm

            def mm_bias(ps=ps, t_tile=t_tile):
                nc.tensor.matmul(
                    ps[:], ones_sb[0:1, :], bv_sb[0:1, :], start=False, stop=True
                )
                dst = V_sb[t_tile].rearrange("p (h e) -> p h e", e=65)[:, :, 0:64]
                src = ps.rearrange("p (h d) -> p h d", d=D)
                nc.vector.tensor_copy(dst, src)

            yield mm_bias

        def gen_out_proj_part(tc_, p, dma_split=False, act_copy=False, pool=None):
            """Out-projection for t_tile 4*tc_+p, both 512-wide n chunks.

            act_copy: evacuate PSUM via the ACT engine instead of DVE. Used in
            the epilogue, where the in-order DVE queue is blocked behind the
            final normalization chain — DVE-side copies would stall PSUM slot
            recycling (and thus the PE) on work that is otherwise ready.
            """
            tt = 4 * tc_ + p
            tsl = slice(tt * NP, (tt + 1) * NP)
            ysb = ypool.tile([NP, C], F32, name="ysb", tag="ysb")
            for nck in range(2):
                nsl = slice(nck * QC, (nck + 1) * QC)
                po = pool or psP
                ps = po.tile(
                    [NP, QC], F32, name="ps_proj",
                    tag="O" if po is psO else "ps_proj",
                )
                for pair in range(NPAIR):
                    last = pair == NPAIR - 1

                    def mm(ps=ps, pair=pair, last=last, tsl=tsl, nsl=nsl, nck=nck):
                        nc.tensor.matmul(
                            ps[:],
                            ON_sb[pair][:, tsl],
                            Wo_sb[:, pair * C + nsl.start : pair * C + nsl.stop],
                            start=(pair == 0),
                            stop=last,
                        )
                        if last:
                            # dma_split (windowed quads): alternate ACT/DVE so
                            # consecutive windows' PSUM slots recycle via
                            # independent engine queues
                            if act_copy and (nck == 0 or not dma_split):
                                nc.scalar.copy(ysb[:, nsl], ps[:])
                            else:
                                nc.vector.tensor_copy(ysb[:, nsl], ps[:])
                            if dma_split:
                                nc.sync.dma_start(y_d[tsl, nsl], ysb[:, nsl])
                            elif nck == 1:
                                nc.sync.dma_start(y_d[tsl, :], ysb[:])

                    yield mm

        # ---------------- filler queues ----------------
        # proj_q: ordered projection work for chunks 0..3, drained just-in-time
        # before the attention unit that needs it, or spliced early between
        # QK/AV matmuls to keep the PE busy while ACT computes exp.
        # op_q: out-projection work, gated per chunk (eligible once the
        # chunk's attention is fully normalized); spliced into late units
        # where projection filler has run out.
        proj_q = []    # items: (chunk, closure)
        mark_qk = {}   # (qc, pair) -> proj_q index that must be drained first
        mark_av = {}   # qc -> proj_q index that must be drained before AV

        for c in range(NQC):
            if c > 0:
                def dma_c(c=c):
                    emit_x_dma(c)

                proj_q.append((c, dma_c))
            for p in range(NPAIR):
                proj_q.extend((c, f) for f in gen_qk_proj_part(c, p))
                mark_qk[(c, p)] = len(proj_q)
                if p == 0:
                    for ts in range(NPAIR):
                        proj_q.extend((c, f) for f in gen_v_proj_part(c, ts))
                    mark_av[c] = len(proj_q)

        op_q = []      # eligible out-proj closures (appended as chunks finish)

        state = {"pq": 0, "qc": 0}

        def drain_to(idx):
            while state["pq"] < idx:
                proj_q[state["pq"]][1]()
                state["pq"] += 1

        def splice(n):
            # pop projection filler, but never front-run more than one chunk
            # ahead of the current attention chunk (preserves filler for the
            # ACT-bound final chunk)
            k = 0
            while (
                k < n
                and state["pq"] < len(proj_q)
                and proj_q[state["pq"]][0] <= state["qc"] + 1
            ):
                proj_q[state["pq"]][1]()
                state["pq"] += 1
                k += 1
            if k == 0 and op_q:
                # ration out-proj filler (1 per 2 slots) so it lasts through
                # the ACT-bound final chunk
                state["tick"] = state.get("tick", 0) + 1
                if state["tick"] % 2 == 0:
                    op_q.pop(0)()

        # ---------------- attention with interleaved filler ----------------
        for qc in range(NQC):
            state["qc"] = qc
            kmax = 4 * (qc + 1)
            qsl = slice(qc * QC, (qc + 1) * QC)
            for pair in range(NPAIR):
                drain_to(mark_qk[(qc, pair)])
                es = []
                for kt in range(kmax):
                    # diagonal tiles (kt >= 4*qc) only need the q-suffix
                    # [off, 512): columns below are fully causal-masked
                    off = max(0, (kt - 4 * qc) * NP)
                    ksl = slice(kt * NP, (kt + 1) * NP)
                    S2 = psS.tile([NP, 2 * QC], F32, name="S2", tag="S2")
                    for hp in range(2):
                        psl = slice(hp * 64, (hp + 1) * 64)
                        nc.tensor.matmul(
                            S2[:, hp * QC + off : (hp + 1) * QC],
                            KT_sb[pair][psl, ksl],
                            QT_sb[pair][psl, qc * QC + off : (qc + 1) * QC],
                            start=True,
                            stop=True,
                            tile_position=(hp * 64, 0),
                        )
                    e2 = epool.tile(
                        [NP, 2 * QC], BF16, name=f"e{kt}", tag=f"e{kt}"
                    )
                    s3 = S2.rearrange("p (h f) -> p h f", h=2)[:, :, off:]
                    e3 = e2.rearrange("p (h f) -> p h f", h=2)[:, :, off:]
                    nc.scalar.activation(
                        e3, s3, mybir.ActivationFunctionType.Exp, scale=SCALE
                    )
                    if off or kt == 4 * qc:  # diagonal: mask the partial block
                        eb = e2.rearrange("p (h f) -> p h f", h=2)[
                            :, :, off : off + NP
                        ]
                        nc.vector.tensor_mul(
                            eb, eb, mask_sb.rearrange("p (h f) -> p h f", h=2)
                        )
                    es.append(e2)
                    # splice filler every 2nd kt (matches the 2-tile S2
                    # pipeline depth) to halve PE array row-mode switches
                    if kt % 2 == 1 or kt == kmax - 1:
                        splice(4)
                drain_to(mark_av[qc])
                for hp in range(2):
                    h = pair * 2 + hp
                    O = psO.tile([NP, QC], F32, name="O", tag="O")
                    for kt in range(kmax):
                        off = max(0, (kt - 4 * qc) * NP)
                        nc.tensor.matmul(
                            O[0:65, off:],
                            V_sb[kt][:, h * 65 : h * 65 + 65],
                            es[kt][:, hp * QC + off : (hp + 1) * QC],
                            start=(kt == 0),
                            stop=(kt == kmax - 1),
                        )
                        splice(1)
                    zinv = spool.tile([NP, QC], F32, name="zinv", tag="zinv")
                    nc.vector.reciprocal(zinv[0:1, :], O[64:65, :])
                    zb = spool.tile([NP, QC], F32, name="zb", tag="zb")
                    nc.gpsimd.partition_broadcast(zb[0:64, :], zinv[0:1, :])
                    nc.vector.tensor_mul(
                        ON_sb[pair][hp * 64 : (hp + 1) * 64, qsl],
                        O[0:64, :],
                        zb[0:64, :],
                    )
                if pair == NPAIR - 1 and qc < NQC - 1:
                    # this chunk's ON is complete: its out-proj becomes eligible.
                    # Hold back two chunk-2 parts as an epilogue reserve: they
                    # depend only on chunk-2 data, so the PE can run them while
                    # the very last normalization chain completes.
                    held = (2, 3) if qc == 2 else ()
                    for p in range(NPAIR):
                        if p not in held or qc != 2:
                            op_q.extend(gen_out_proj_part(qc, p))
                    if qc == 2:
                        reserve = [
                            f
                            for p in held
                            for f in gen_out_proj_part(qc, p, act_copy=True)
                        ]

        # ---------------- epilogue ----------------
        drain_to(len(proj_q))
        while op_q:
            op_q.pop(0)()
        for f in reserve:
            f()
        # Final chunk's out-projection: emit two groups' pair-0..2 matmuls
        # (which only need already-normalized pairs) ahead of either group's
        # pair-3 matmul, so the PE works through the tail of the last
        # normalization chain instead of stalling on it.
        groups = [
            list(
                gen_out_proj_part(
                    NQC - 1,
                    p,
                    dma_split=True,
                    act_copy=True,
                    pool=(psP if p % 2 == 0 else psO),
                )
            )
            for p in range(NPAIR)
        ]
        # each gen yields 8 mms = 2 psum groups of 4 (nck 0 and 1)
        quads = [g[i : i + 4] for g in groups for i in (0, 4)]
        for w in range(0, len(quads), 2):
            ga, gb = quads[w], quads[w + 1]
            for mm in ga[:3]:
                mm()
            for mm in gb[:3]:
                mm()
            ga[3]()
            gb[3]()

    nc.compile()
    return nc


def get_nc():
    if "nc" not in _CACHE:
        _CACHE["nc"] = _build()
    return _CACHE["nc"]


def make_in_maps(k, v, q, Wq, bq, Wk, bk, Wv, bv, Wo, bo):
    k = np.asarray(k, np.float32)
    v = np.asarray(v, np.float32)
    q = np.asarray(q, np.float32)
    Wq = np.asarray(Wq, np.float32).astype(BF)
    Wk = np.asarray(Wk, np.float32).astype(BF)
    Wv = np.asarray(Wv, np.float32).astype(BF)
    Wo = np.asarray(Wo, np.float32).astype(BF)
    bq = np.asarray(bq, np.float32)
    bk = np.asarray(bk, np.float32)
    bv = np.asarray(bv, np.float32).astype(BF)
    in_maps = []
    for core in range(8):
        b, hh = core // 2, core % 2
        sl = slice(hh * CS, (hh + 1) * CS)
        in_maps.append(
            {
                "qT": np.ascontiguousarray(q[b].T.astype(BF)),
                "kT": np.ascontiguousarray(k[b].T.astype(BF)),
                "vT": np.ascontiguousarray(v[b].T.astype(BF)),
                "Wq": np.ascontiguousarray(Wq[:, sl]),
                "Wk": np.ascontiguousarray(Wk[:, sl]),
                "Wv": np.ascontiguousarray(Wv[:, sl]),
                "Wo": np.ascontiguousarray(Wo[sl, :]),
                "bq": np.ascontiguousarray(bq[sl]),
                "bk": np.ascontiguousarray(bk[sl]),
                "bv": np.ascontiguousarray(bv[sl]),
            }
        )
    return in_maps


def kernel(k, v, q, Wq, bq, Wk, bk, Wv, bv, Wo, bo):
    nc = get_nc()
    in_maps = make_in_maps(k, v, q, Wq, bq, Wk, bk, Wv, bv, Wo, bo)
    res = bass_utils.run_bass_kernel_spmd(nc, in_maps, core_ids=list(range(8)))
    bo = np.asarray(bo, np.float32)
    out = np.empty((B, T, C), np.float32)
    for b in range(B):
        out[b] = res.results[2 * b]["y"] + res.results[2 * b + 1]["y"] + bo
    return out
